# revision 1
# baseline (speedup 1.0000x reference)
"""Trainium2 Bass kernel for nn_BSplineActivation.

Math: y[b,f] = sum_n B_n(x[b,f]) * coeff[f,n] with cubic B-spline bases on a
uniform grid linspace(-1,1,14). Using the truncated-power identity
  M3(v) = (1/6) sum_r (-1)^r C(4,r) (v-r)_+^3
the whole activation collapses to
  y = sum_{j=0}^{12} d_j[f] * relu(u - j)^3,   u = 6.5*clip(x,-1,1) + 6.5
with d_j[f] = (1/6) sum_r (-1)^r C(4,r) coeff[f, j-r].

Per j:  R_j = relu(u - j),  S_j = R_j^2 (ACT Square), then with
  y = (u - 6.5)*A + C,  A = sum d_j S_j,  C = sum (6.5-j) d_j S_j
the (A, C) accumulation chains are split by data chunk: PE chunks use
diagonal-matmul PSUM accumulation (lhsT = diag(d_j), exact fp32), the rest
use DVE scalar_tensor_tensor chains with per-partition scalar columns.
R_j producers are balanced across POOL/ACT; diag matrices are built by POOL
affine_select from a packed per-partition table; the span tail add
(Y += Cacc) rides a SWDGE DMA with destination-accumulate, freeing DVE.

Device layout: features on partitions (128 per group, 8 groups/core), batch
along the free dim; pure data parallel over batch across 8 cores. The host
passes per-core batch shards transposed (features-major) so all DMAs are
burst-friendly; total HBM bytes moved are identical to the untransposed
layout.
"""

import os
from math import comb

import numpy as np

import concourse.bacc as bacc
import concourse.bass as bass
import concourse.mybir as mybir
import concourse.tile as tile
from concourse.bass_utils import run_bass_kernel_spmd

N_CORES = 8
B_FULL, F = 8192, 1024
B_CORE = B_FULL // N_CORES  # 1024
NB = 13
P = 128
G = F // P  # 8
QUARTERS = 4
CHUNK = 512
FP32 = mybir.dt.float32

Alu = mybir.AluOpType
Act = mybir.ActivationFunctionType

# PE owns chunks [0, PE_SPAN); the rest is the DVE dual-chain span.
PE_CHUNKS = [(0, 0), (0, 1)]  # contiguous span 0..1024
PE_SPAN = 2 * CHUNK
# chunk (0,1): j < MIX_SPLIT accumulate on PE, j >= MIX_SPLIT on DVE
# (partial sums merged in the tail).
MIX_SPLIT = 10
# R_j producer per j: "dve" | "pool" | "act" (R_0 = u is skipped entirely)
RENG = {
    0: "pool", 1: "act", 2: "pool", 3: "act", 4: "pool", 5: "act",
    6: "pool", 7: "act", 8: "pool", 9: "act", 10: "pool", 11: "act", 12: "pool",
}
_CACHE: dict = {}


def _build_nc() -> bass.Bass:
    nc = bacc.Bacc("TRN2", target_bir_lowering=False, debug=False)

    xT = nc.dram_tensor("xT", [F, B_CORE], FP32, kind="ExternalInput")
    # packed tables: cols [0, G*NB) = d_j; [G*NB, 2*G*NB) = c_j = (6.5-j)*d_j;
    # cols [2*G*NB, 2*G*NB+NB) = constant -j (ACT relu bias columns)
    tabs = nc.dram_tensor("tabs", [P, 2 * G * NB + NB], FP32, kind="ExternalInput")
    yT = nc.dram_tensor("yT", [F, B_CORE], FP32, kind="ExternalOutput")

    W = 2 * B_CORE

    with tile.TileContext(nc) as tc:
        with (
            tc.tile_pool(name="const", bufs=1) as const_pool,
            tc.tile_pool(name="xdata", bufs=2) as x_pool,
            tc.tile_pool(name="rs", bufs=4) as rs_pool,
            tc.tile_pool(name="yout", bufs=2) as y_pool,
            tc.tile_pool(name="diag", bufs=2) as diag_pool,
            tc.tile_pool(name="psum", bufs=2, space="PSUM") as psum_pool,
        ):
            tabs_t = const_pool.tile([P, 2 * G * NB + NB], FP32, name="tabs_t")
            nc.sync.dma_start(tabs_t[:], tabs[:])

            def dcol(g, j):
                return tabs_t[:, g * NB + j : g * NB + j + 1]

            def ccol(g, j):
                c = G * NB + g * NB + j
                return tabs_t[:, c : c + 1]

            def bcol(j):
                c = 2 * G * NB + j
                return tabs_t[:, c : c + 1]

            for q in range(QUARTERS):
                g0 = 2 * q
                X = x_pool.tile([P, W], FP32, name="X", tag="X")
                if q == 0:
                    # split the first quarter's preamble into slivers so the
                    # PE's first matmul (needs S_0[:, 0:CHUNK] only) starts
                    # ~6us sooner — this is the pipeline-fill critical path
                    nc.sync.dma_start(
                        X[:, :CHUNK], xT[g0 * P : (g0 + 1) * P, :CHUNK]
                    )
                    nc.sync.dma_start(
                        X[:, CHUNK:B_CORE],
                        xT[g0 * P : (g0 + 1) * P, CHUNK:B_CORE],
                    )
                    nc.sync.dma_start(
                        X[:, B_CORE:W], xT[(g0 + 1) * P : (g0 + 2) * P, :]
                    )
                    for sl in (slice(0, CHUNK), slice(CHUNK, W)):
                        nc.vector.tensor_scalar(
                            X[:, sl], X[:, sl], -1.0, 1.0, Alu.max, Alu.min
                        )
                        nc.vector.tensor_scalar(
                            X[:, sl], X[:, sl], 6.5, 6.5, Alu.mult, Alu.add
                        )
                else:
                    nc.sync.dma_start(
                        X[:].rearrange("p (gl b) -> p gl b", gl=2),
                        xT[g0 * P : (g0 + 2) * P, :].rearrange(
                            "(gl p) b -> p gl b", p=P
                        ),
                    )
                    nc.vector.tensor_scalar(X[:], X[:], -1.0, 1.0, Alu.max, Alu.min)
                    nc.vector.tensor_scalar(X[:], X[:], 6.5, 6.5, Alu.mult, Alu.add)

                pe_gls = sorted({gl for (gl, ch) in PE_CHUNKS})
                diagsA = {}
                diagsC = {}
                for gl in pe_gls:
                    g = g0 + gl
                    for j in range(NB):
                        dA = diag_pool.tile(
                            [P, P], FP32, name=f"dA{gl}_{j}", tag=f"dA{gl}_{j}"
                        )
                        dC = diag_pool.tile(
                            [P, P], FP32, name=f"dC{gl}_{j}", tag=f"dC{gl}_{j}"
                        )
                        nc.gpsimd.affine_select(
                            dA[:], dcol(g, j).broadcast_to([P, P]),
                            pattern=[[-1, P]], compare_op=Alu.is_equal,
                            fill=0.0, base=0, channel_multiplier=1,
                        )
                        nc.gpsimd.affine_select(
                            dC[:], ccol(g, j).broadcast_to([P, P]),
                            pattern=[[-1, P]], compare_op=Alu.is_equal,
                            fill=0.0, base=0, channel_multiplier=1,
                        )
                        diagsA[(gl, j)] = dA
                        diagsC[(gl, j)] = dC

                Y = y_pool.tile([P, W], FP32, name="Y", tag="Y")
                Apsum = {}
                Cpsum = {}
                for key in PE_CHUNKS:
                    gl, ch = key
                    Apsum[key] = psum_pool.tile(
                        [P, CHUNK], FP32, name=f"Yp{gl}{ch}", tag=f"Yp{gl}{ch}"
                    )
                    Cpsum[key] = psum_pool.tile(
                        [P, CHUNK], FP32, name=f"Cq{gl}{ch}", tag=f"Cq{gl}{ch}"
                    )
                # dual-chain accumulators for the DVE span [PE_SPAN, W)
                DW = W - PE_SPAN
                Aacc = y_pool.tile([P, DW], FP32, name="Aacc", tag="Aacc")
                Cacc = y_pool.tile([P, DW], FP32, name="Cacc", tag="Cacc")
                # DVE partials for the mixed chunk (0,1), j >= MIX_SPLIT
                Amix = y_pool.tile([P, CHUNK], FP32, name="Amix", tag="Amix")
                Cmix = y_pool.tile([P, CHUNK], FP32, name="Cmix", tag="Cmix")

                for j in range(NB):
                    if j == 0:
                        R = X  # relu(u - 0) = u since u >= 0
                    else:
                        R = rs_pool.tile([P, W], FP32, name="R", tag="R")
                        reng = RENG[j]
                        if reng == "dve":
                            nc.vector.tensor_scalar(
                                R[:], X[:], float(-j), 0.0, Alu.add, Alu.max
                            )
                        elif reng == "pool":
                            nc.gpsimd.tensor_scalar(
                                R[:], X[:], float(-j), 0.0, Alu.add, Alu.max
                            )
                        else:
                            nc.scalar.activation(
                                R[:], X[:], Act.Relu, bias=bcol(j), scale=1.0
                            )
                    S = rs_pool.tile([P, W], FP32, name="S", tag="S")
                    if q == 0 and j == 0:
                        nc.scalar.activation(S[:, :CHUNK], R[:, :CHUNK], Act.Square)
                        nc.scalar.activation(S[:, CHUNK:], R[:, CHUNK:], Act.Square)
                    else:
                        nc.scalar.activation(S[:], R[:], Act.Square)
                    for gl, ch in PE_CHUNKS:
                        if (gl, ch) == (0, 1) and j >= MIX_SPLIT:
                            continue  # handled by the DVE mix chain below
                        lo = gl * B_CORE + ch * CHUNK
                        last = (j == NB - 1) if (gl, ch) != (0, 1) else (
                            j == MIX_SPLIT - 1
                        )
                        nc.tensor.matmul(
                            Apsum[(gl, ch)][:], diagsA[(gl, j)][:],
                            S[:, lo : lo + CHUNK],
                            start=(j == 0), stop=last,
                        )
                        nc.tensor.matmul(
                            Cpsum[(gl, ch)][:], diagsC[(gl, j)][:],
                            S[:, lo : lo + CHUNK],
                            start=(j == 0), stop=last,
                        )
                    if j >= MIX_SPLIT:
                        # DVE partial for mixed chunk (0,1): features of gl=0
                        g = g0
                        msl = S[:, CHUNK : 2 * CHUNK]
                        if j == MIX_SPLIT:
                            nc.vector.tensor_scalar(
                                Amix[:], msl, dcol(g, j), None, Alu.mult
                            )
                            nc.vector.tensor_scalar(
                                Cmix[:], msl, ccol(g, j), None, Alu.mult
                            )
                        else:
                            nc.vector.scalar_tensor_tensor(
                                Amix[:], msl, dcol(g, j), Amix[:], Alu.mult, Alu.add
                            )
                            nc.vector.scalar_tensor_tensor(
                                Cmix[:], msl, ccol(g, j), Cmix[:], Alu.mult, Alu.add
                            )
                    # dual chain on S for the tail span (features of gl=1)
                    g = g0 + 1
                    ssl = S[:, PE_SPAN:W]
                    if j == 0:
                        nc.vector.tensor_scalar(
                            Aacc[:], ssl, dcol(g, j), None, Alu.mult
                        )
                        nc.vector.tensor_scalar(
                            Cacc[:], ssl, ccol(g, j), None, Alu.mult
                        )
                    else:
                        nc.vector.scalar_tensor_tensor(
                            Aacc[:], ssl, dcol(g, j), Aacc[:], Alu.mult, Alu.add
                        )
                        nc.vector.scalar_tensor_tensor(
                            Cacc[:], ssl, ccol(g, j), Cacc[:], Alu.mult, Alu.add
                        )
                # dual-span tail first: it depends only on the DVE chains
                # (not the PE), so it must not queue behind the PSUM drains
                nc.vector.scalar_tensor_tensor(
                    Y[:, PE_SPAN:W], X[:, PE_SPAN:W],
                    -6.5, Aacc[:], Alu.add, Alu.mult,
                )
                # Y += Cacc via SWDGE destination-accumulate (offloads DVE)
                nc.gpsimd.dma_start(
                    Y[:, PE_SPAN:W], Cacc[:], accum_op=Alu.add
                )
                # merge mixed-chunk DVE partials into its psum result
                # (DVE: GPSIMD has no PSUM port)
                nc.vector.tensor_tensor(
                    Amix[:], Amix[:], Apsum[(0, 1)][:], Alu.add
                )
                nc.vector.tensor_tensor(
                    Cmix[:], Cmix[:], Cpsum[(0, 1)][:], Alu.add
                )

                # tail: PE chunks drain psum -> Y on ACT; DVE span computes
                # y = (u - 6.5) * A + C in place into Y
                # fused tail: Y = (X - 6.5) * A in one scalar_tensor_tensor
                # (0,1) first: its psum group stops at j=9 and the mix
                # chains finish before the (0,0) group's j=12 matmuls
                for gl, ch in reversed(PE_CHUNKS):
                    lo = gl * B_CORE + ch * CHUNK
                    Afin = Amix[:] if (gl, ch) == (0, 1) else Apsum[(gl, ch)][:]
                    Cfin = Cmix[:] if (gl, ch) == (0, 1) else Cpsum[(gl, ch)][:]
                    nc.vector.scalar_tensor_tensor(
                        Y[:, lo : lo + CHUNK], X[:, lo : lo + CHUNK],
                        -6.5, Afin, Alu.add, Alu.mult,
                    )
                    nc.vector.tensor_tensor(
                        Y[:, lo : lo + CHUNK], Y[:, lo : lo + CHUNK],
                        Cfin, Alu.add,
                    )
                    # ship each finished chunk of the last quarter at once
                    if q == QUARTERS - 1:
                        nc.sync.dma_start(
                            yT[g0 * P : (g0 + 1) * P, lo : lo + CHUNK],
                            Y[:, lo : lo + CHUNK],
                        )
                # split out-DMA: the PE-span half of Y is done before the
                # SWDGE Cacc-add lands on the other half
                if q != QUARTERS - 1:
                    nc.sync.dma_start(
                        yT[g0 * P : (g0 + 1) * P, :], Y[:, :B_CORE]
                    )
                nc.sync.dma_start(
                    yT[(g0 + 1) * P : (g0 + 2) * P, :], Y[:, B_CORE:W]
                )
    nc.compile()
    return nc


def _tables(coeff: np.ndarray):
    """Packed [P, 2*G*NB + NB] fp32 table: d_j, c_j, -j bias columns."""
    d = np.zeros((NB, F), dtype=np.float64)
    c64 = coeff.astype(np.float64)
    for j in range(NB):
        for r in range(5):
            n = j - r
            if 0 <= n < coeff.shape[1]:
                d[j] += (-1) ** r * comb(4, r) / 6.0 * c64[:, n]
    c = (6.5 - np.arange(NB))[:, None] * d
    dt = d.astype(np.float32).T.reshape(G, P, NB).transpose(1, 0, 2).reshape(P, G * NB)
    ct = c.astype(np.float32).T.reshape(G, P, NB).transpose(1, 0, 2).reshape(P, G * NB)
    bt = np.broadcast_to(-np.arange(NB, dtype=np.float32), (P, NB))
    return np.ascontiguousarray(np.concatenate([dt, ct, bt], axis=1))


def kernel(x: np.ndarray, coeff: np.ndarray) -> np.ndarray:
    x = np.ascontiguousarray(x, dtype=np.float32)
    coeff = np.ascontiguousarray(coeff, dtype=np.float32)
    assert x.shape == (B_FULL, F) and coeff.shape == (F, 10)

    if "nc" not in _CACHE:
        _CACHE["nc"] = _build_nc()
    nc = _CACHE["nc"]

    tabs = _tables(coeff)

    in_maps = []
    for c in range(N_CORES):
        shard = np.ascontiguousarray(x[c * B_CORE : (c + 1) * B_CORE, :].T)
        in_maps.append({"xT": shard, "tabs": tabs})

    trace = os.environ.get("BSPLINE_TRACE", "0") == "1"
    res = run_bass_kernel_spmd(
        nc, in_maps, core_ids=list(range(N_CORES)), trace=trace
    )
    _CACHE["last_result"] = res

    y = np.empty((B_FULL, F), dtype=np.float32)
    for c in range(N_CORES):
        y[c * B_CORE : (c + 1) * B_CORE, :] = res.results[c]["yT"].T
    return y



# revision 2
# speedup vs baseline: 1.9303x; 1.9303x over previous
"""Trainium2 Bass kernel for nn_BSplineActivation (reflected truncated-power form).

Math: y[b,f] = sum_n B_n(x[b,f]) coeff[f,n], cubic B-splines on the uniform
grid linspace(-1,1,14).  In truncated-power form with u = 6.5(clip(x)+1):
  y = sum_{j=0..12} d_j (u-j)_+^3.
Adding the j=13 term (d_13 = coeff_9/6, zero on u<13) makes
  p(u) = sum_{j=0..13} d_j (u-j)^3 == 0  identically, so for u>6.5 the sum
collapses to minus-side powers of the *reflected* coordinate.  With
ax = |clip(x)| (= min(|x|,1)) and b~_m = m/6.5 for m in {0.5,...,6.5}:
  K_m = min((ax - b~_m)^3, 0)        -- 7 shared planes for BOTH branches
  x <  0:  y = sum_m gA_m K_m        (gA_m = -6.5^3 d_{6.5-m})
  x >= 0:  y = sum_m gB_m K_m        (gB_m = -6.5^3 d_{6.5+m})
Two PSUM chains (P = gB-weighted, M = gA-weighted) of per-feature diagonal
matmuls accumulate both branches; the finish selects P or M by sign(x).

Precision: planes m<=2.5 (small |gK| bound b~^3) run in fp16 end-to-end with
fp16 diagonal weights; planes m>=3.5 run fp32 with fp32r (tf32-rounded)
matmuls.  Simulated rel-l2 vs the reference: ~1.6e-2 (< 2e-2 gate).

Layout: features on partitions (8 groups of 128), batch on the free dim,
pure data-parallel over batch across 8 cores.  Host sends per-core shards
transposed and cast to fp16 (halves input DMA), and pre-builds all diagonal
weight tiles (zero engine cost for diag construction, ~5.7MB extra DMA).
"""

import os
from math import comb

import numpy as np

import concourse.bacc as bacc
import concourse.bass as bass
import concourse.mybir as mybir
import concourse.tile as tile
from concourse.bass_utils import run_bass_kernel_spmd

N_CORES = 8
B_FULL, F = 8192, 1024
B_CORE = B_FULL // N_CORES  # 1024
P = 128
G = F // P  # 8
CHUNK = 512
NPLANES = 7
BT = [(2 * i + 1) / 13.0 for i in range(NPLANES)]  # (m=i+0.5)/6.5
FP16_PLANES = (0, 1, 2)   # fp16 planes (small-magnitude terms)
FP32_PLANES = (3, 4, 5, 6)

FP32 = mybir.dt.float32
FP16 = mybir.dt.float16
F32R = mybir.dt.float32r
U8 = mybir.dt.uint8

Alu = mybir.AluOpType
Act = mybir.ActivationFunctionType

# Engine routing per fp32 plane: (N-producer, S-producer, K-producer)
#   N: "dve" ts | "pool" ts          (N = min(ax-bt,0))
#   S: "act" ungated Square(bt-ax)   (only choice; independent of N)
#   K: "dve" tt | "pool" tt          (K = S*N, written as fp32r)
ROUTE32 = {
    3: ("dve", "act", "dve"),
    4: ("dve", "act", "dve"),
    5: ("pool", "act", "pool"),
    6: ("pool", "act", "pool"),
}
_CACHE: dict = {}


def _build_nc() -> bass.Bass:
    nc = bacc.Bacc("TRN2", target_bir_lowering=False, debug=False)

    xT = nc.dram_tensor("xT", [F, B_CORE], FP16, kind="ExternalInput")
    # host-packed diagonal weight tiles, partition-major:
    # d16[g]: [128, 6*128] fp16  (P,M interleaved per plane i=0,1,2)
    # d32[g]: [128, 8*128] fp32r (P,M per plane i=3..6)
    d16 = nc.dram_tensor("d16", [G, P, 6 * P], FP16, kind="ExternalInput")
    d32 = nc.dram_tensor("d32", [G, P, 8 * P], F32R, kind="ExternalInput")
    # bias columns for ACT Square: col i = BT[3+i]
    cst = nc.dram_tensor("cst", [P, 4], FP32, kind="ExternalInput")
    yT = nc.dram_tensor("yT", [F, B_CORE], FP16, kind="ExternalOutput")

    W = B_CORE  # 1024 free-dim columns per group

    with tile.TileContext(nc) as tc:
        with (
            tc.tile_pool(name="const", bufs=1) as const_pool,
            tc.tile_pool(name="wts", bufs=2) as w_pool,
            tc.tile_pool(name="xdata", bufs=2) as x_pool,
            tc.tile_pool(name="plane", bufs=2) as pl_pool,
            tc.tile_pool(name="yout", bufs=2) as y_pool,
            tc.tile_pool(name="psum", bufs=2, space="PSUM") as psum_pool,
        ):
            ct = const_pool.tile([P, 4], FP32, name="ct")
            nc.sync.dma_start(ct[:], cst[:])

            for g in range(G):
                x16 = x_pool.tile([P, W], FP16, name="x16", tag="x16")
                nc.sync.dma_start(x16[:], xT[g * P : (g + 1) * P, :])
                dg16 = w_pool.tile([P, 6 * P], FP16, name="dg16", tag="dg16")
                nc.sync.dma_start(dg16[:], d16[g])
                dg32 = w_pool.tile([P, 8 * P], F32R, name="dg32", tag="dg32")
                nc.sync.dma_start(dg32[:], d32[g])

                def dP(i):
                    if i in FP16_PLANES:
                        return dg16[:, (2 * i) * P : (2 * i + 1) * P]
                    k = i - 3
                    return dg32[:, (2 * k) * P : (2 * k + 1) * P]

                def dM(i):
                    if i in FP16_PLANES:
                        return dg16[:, (2 * i + 1) * P : (2 * i + 2) * P]
                    k = i - 3
                    return dg32[:, (2 * k + 1) * P : (2 * k + 2) * P]

                ax = pl_pool.tile([P, W], FP16, name="ax", tag="ax")
                nc.scalar.activation(ax[:], x16[:], Act.Abs)

                K = {}
                # fp16 planes: N,S,K all DVE
                for i in FP16_PLANES:
                    n = pl_pool.tile([P, W], FP16, name=f"n{i}", tag=f"n{i}")
                    nc.vector.tensor_scalar(
                        n[:], ax[:], BT[i], 0.0, Alu.subtract, Alu.min
                    )
                    s = pl_pool.tile([P, W], FP16, name=f"s{i}", tag=f"s{i}")
                    nc.vector.tensor_tensor(s[:], n[:], n[:], Alu.mult)
                    k = pl_pool.tile([P, W], FP16, name=f"k{i}", tag=f"k{i}")
                    nc.vector.tensor_tensor(k[:], s[:], n[:], Alu.mult)
                    K[i] = k
                # fp32 planes
                for i in FP32_PLANES:
                    rn, rs, rk = ROUTE32[i]
                    n = pl_pool.tile([P, W], FP32, name=f"n{i}", tag=f"n{i}")
                    eng = nc.vector if rn == "dve" else nc.gpsimd
                    eng.tensor_scalar(n[:], ax[:], BT[i], 0.0, Alu.subtract, Alu.min)
                    s = pl_pool.tile([P, W], FP32, name=f"s{i}", tag=f"s{i}")
                    # ungated square: (bt - ax)^2, direct from ax
                    nc.scalar.activation(
                        s[:], ax[:], Act.Square, bias=ct[:, i - 3 : i - 2], scale=-1.0
                    )
                    k = pl_pool.tile([P, W], F32R, name=f"k{i}", tag=f"k{i}")
                    eng = nc.vector if rk == "dve" else nc.gpsimd
                    eng.tensor_tensor(k[:], s[:], n[:], Alu.mult)
                    K[i] = k

                # chains: per 512-chunk, P and M PSUM accumulations
                Pp = {}
                Mp = {}
                for ch in range(W // CHUNK):
                    Pp[ch] = psum_pool.tile([P, CHUNK], FP32, name=f"Pp{ch}", tag=f"Pp{ch}")
                    Mp[ch] = psum_pool.tile([P, CHUNK], FP32, name=f"Mp{ch}", tag=f"Mp{ch}")
                for i in range(NPLANES):
                    for ch in range(W // CHUNK):
                        sl = K[i][:, ch * CHUNK : (ch + 1) * CHUNK]
                        nc.tensor.matmul(
                            Pp[ch][:], dP(i), sl, start=(i == 0), stop=(i == NPLANES - 1)
                        )
                        nc.tensor.matmul(
                            Mp[ch][:], dM(i), sl, start=(i == 0), stop=(i == NPLANES - 1)
                        )

                # finish: y = (x >= 0) ? P : M
                g8 = pl_pool.tile([P, W], U8, name="g8", tag="g8")
                nc.vector.tensor_scalar(g8[:], x16[:], 0.0, 1.0, Alu.is_ge, Alu.mult)
                y16 = y_pool.tile([P, W], FP16, name="y16", tag="y16")
                for ch in range(W // CHUNK):
                    sl = slice(ch * CHUNK, (ch + 1) * CHUNK)
                    nc.scalar.copy(y16[:, sl], Mp[ch][:])
                    nc.vector.copy_predicated(y16[:, sl], g8[:, sl], Pp[ch][:])
                nc.sync.dma_start(yT[g * P : (g + 1) * P, :], y16[:])
    nc.compile()
    return nc


def _weights(coeff: np.ndarray):
    """gA/gB [7, F] fp64 and packed diag DRAM arrays."""
    d = np.zeros((14, F), dtype=np.float64)
    c64 = coeff.astype(np.float64)
    for j in range(14):
        for r in range(5):
            n = j - r
            if 0 <= n < 10:
                d[j] += (-1) ** r * comb(4, r) / 6.0 * c64[:, n]
    s = 6.5 ** 3
    gA = np.zeros((NPLANES, F))
    gB = np.zeros((NPLANES, F))
    for i in range(NPLANES):
        gA[i] = -s * d[6 - i]       # j = 6.5 - m
        gB[i] = -s * d[7 + i]       # j = 6.5 + m
    # d16[g]: [128, 6*128] fp16: per plane i: [diag(gB_i) | diag(gA_i)]
    d16 = np.zeros((G, P, 6 * P), dtype=np.float16)
    d32 = np.zeros((G, P, 8 * P), dtype=np.float32)
    for g in range(G):
        fsl = slice(g * P, (g + 1) * P)
        for i in FP16_PLANES:
            d16[g, :, (2 * i) * P : (2 * i + 1) * P][np.arange(P), np.arange(P)] = (
                gB[i, fsl].astype(np.float16)
            )
            d16[g, :, (2 * i + 1) * P : (2 * i + 2) * P][np.arange(P), np.arange(P)] = (
                gA[i, fsl].astype(np.float16)
            )
        for i in FP32_PLANES:
            k = i - 3
            d32[g, :, (2 * k) * P : (2 * k + 1) * P][np.arange(P), np.arange(P)] = (
                gB[i, fsl].astype(np.float32)
            )
            d32[g, :, (2 * k + 1) * P : (2 * k + 2) * P][np.arange(P), np.arange(P)] = (
                gA[i, fsl].astype(np.float32)
            )
    cst = np.broadcast_to(
        np.array(BT[3:], dtype=np.float32), (P, 4)
    ).copy()
    return d16, d32, cst


def kernel(x: np.ndarray, coeff: np.ndarray) -> np.ndarray:
    x = np.ascontiguousarray(x, dtype=np.float32)
    coeff = np.ascontiguousarray(coeff, dtype=np.float32)
    assert x.shape == (B_FULL, F) and coeff.shape == (F, 10)

    if "nc" not in _CACHE:
        _CACHE["nc"] = _build_nc()
    nc = _CACHE["nc"]

    d16, d32, cst = _weights(coeff)

    in_maps = []
    for c in range(N_CORES):
        shard = np.ascontiguousarray(
            x[c * B_CORE : (c + 1) * B_CORE, :].T.astype(np.float16)
        )
        in_maps.append({"xT": shard, "d16": d16, "d32": d32, "cst": cst})

    trace = os.environ.get("BSPLINE_TRACE", "0") == "1"
    res = run_bass_kernel_spmd(
        nc, in_maps, core_ids=list(range(N_CORES)), trace=trace
    )
    _CACHE["last_result"] = res

    y = np.empty((B_FULL, F), dtype=np.float32)
    for c in range(N_CORES):
        y[c * B_CORE : (c + 1) * B_CORE, :] = res.results[c]["yT"].T.astype(np.float32)
    return y
